# revision 15
# baseline (speedup 1.0000x reference)
"""Trainium2 Bass kernel for nn_CustomLoss_43645457662200.

Loss over B=4,194,304 samples:
    lower = pred[:, 0], upper = pred[:, 1], center = (lower+upper)/2
    center_loss  = mean((target - center)^2)
    width_loss   = mean(upper - lower)
    valid_pen    = mean(relu(lower - upper))
    dir_pen      = sum(relu((center - prev) * s)),  s = (1-2*pv) * (dt != 0)
    total = 1.5*center_loss + 0.1*width_loss + 10*valid_pen + 0.5*dir_pen/B

Strategy: pure data-parallel over 8 NeuronCores (524288 samples each).
Per core, tiles of [128, TILE_F] stream through a fused elementwise
pipeline on the Vector/Scalar engines; all global sums are produced
on-chip via fused accum_out reductions (per-partition [128,1] partials
per tile).  Each core emits a tiny [5,128,NT] partial tensor; the final
combine runs on host in float64 using only sums:
    sum(u-l)      = Ssu - 2*Sl          (Ssu = sum(l+u))
    sum(relu(l-u))= Smx - Su            (Smx = sum(max(l,u)))
    sum((t-c)^2)  = Ssq                 (y = 0.5*(l+u) - t, squared)
    dir_pen       = Spen                (relu((c-p)*s), s = min(dt,1)*(1-2pv))

Sync-wait discipline (walrus codegen rejects instructions whose wait
list exceeds the struct's slot budget — STT/TS, activation-with-accum
and pseudo-DMA all have tiny budgets):
  * every cross-proc dependency of an STT/TS is absorbed by a tiny
    single-dest DVE gate copy ordered in front of it (nosync dep edge);
  * dual-dest activations keep only their same-engine WAW wait; their
    cross-proc waits are absorbed by single-dest ScalarE gate copies;
  * tiles read by a foreign engine live in a no-reuse pool (one buffer
    per iteration) so no WAR wait ever lands on their producer;
  * input tiles also get one buffer per iteration so input DMAs carry
    no WAR waits at all.
"""

import numpy as np

from concourse import bass, mybir
from concourse.bass_utils import run_bass_kernel_spmd
from concourse.tile import TileContext
from concourse.tile_rust import add_dep_helper

B = 4_194_304
NCORES = 8
N = B // NCORES  # 524288 samples per core
P = 128
CPT = N // P  # 4096 free-dim columns per core per tensor
TILE_F = 1024

f32 = mybir.dt.float32
i32 = mybir.dt.int32


def _legalize_sync_waits(nc: bass.Bass) -> bass.Bass:
    """Split multi-wait instructions for this walrus build.

    The neuronxcc walrus in this container rejects ANY instruction whose
    sync_info carries more than one wait command ("Too many sync wait
    commands", even for plain TensorTensor — the stock tile_nary_add
    kernel trips it too).  Hoist all but the last wait of each
    instruction onto freshly injected same-engine NoOps placed directly
    before it; engine sequencers execute waits in stream order, so the
    semantics are identical.
    """
    counter = 0
    for fn in nc.m.functions:
        for blk in fn.blocks:
            insts = blk.instructions
            out = []
            changed = False
            for ins in insts:
                si = ins.sync_info
                waits = list(si.on_wait) if si is not None and si.on_wait else []
                if len(waits) > 1:
                    changed = True
                    for w in waits[:-1]:
                        counter += 1
                        nop = mybir.InstNoOp(name=f"waitsplit_{counter}")
                        nop.engine = ins.engine
                        nop.sync_info = mybir.SyncInfo(on_wait=[w], on_update=[])
                        out.append(nop)
                    ins.sync_info = mybir.SyncInfo(
                        on_wait=[waits[-1]], on_update=list(si.on_update or [])
                    )
                out.append(ins)
            if changed:
                blk.instructions = out
    return nc


def build_program(cpt: int = CPT, tile_f: int = TILE_F) -> bass.Bass:
    assert cpt % tile_f == 0
    nt = cpt // tile_f
    Op = mybir.AluOpType
    Act = mybir.ActivationFunctionType

    nc = bass.Bass()
    pred = nc.declare_dram_parameter("pred", [P, 2 * cpt], f32, isOutput=False)
    target = nc.declare_dram_parameter("target", [P, cpt], f32, isOutput=False)
    prev = nc.declare_dram_parameter("prev_pci", [P, cpt], f32, isOutput=False)
    dt_p = nc.declare_dram_parameter("delta_time", [P, cpt], i32, isOutput=False)
    pv_p = nc.declare_dram_parameter("pv_values", [P, cpt], i32, isOutput=False)
    acc_out = nc.declare_dram_parameter("acc_out", [P, 5 * nt], f32, isOutput=True)

    with TileContext(nc) as tc:
        with (
            tc.tile_pool(name="accs", bufs=1) as accpool,
            tc.tile_pool(name="io", bufs=max(nt, 2)) as iopool,
            tc.tile_pool(name="mid", bufs=2) as midpool,
            tc.tile_pool(name="mid_nr", bufs=max(nt, 2)) as midnr,
            tc.tile_pool(name="gates", bufs=max(nt, 2)) as gatepool,
        ):
            su_acc = accpool.tile([P, nt], f32, tag="su_acc")
            l_acc = accpool.tile([P, nt], f32, tag="l_acc")
            mx_acc = accpool.tile([P, nt], f32, tag="mx_acc")
            pen_acc = accpool.tile([P, nt], f32, tag="pen_acc")
            sq_acc = accpool.tile([P, nt], f32, tag="sq_acc")

            # Persistent junk sinks for the dual-dest ops whose primary
            # output is unused.  Persistent => cross-iteration WAW stays
            # same-engine program order (DVE emits no wait for it; the
            # ScalarE ops pay exactly one same-engine wait, which is
            # within the activation struct's budget).
            mxj = accpool.tile([P, tile_f], f32, tag="mxj")
            lj = accpool.tile([P, tile_f], f32, tag="lj")
            sqj = accpool.tile([P, tile_f], f32, tag="sqj")
            penj = accpool.tile([P, tile_f], f32, tag="penj")
            gate_junk = {
                tag: accpool.tile([P, 1], f32, tag=f"gj_{tag}", name=f"gj_{tag}")
                for tag in ("pred", "tt", "pt", "pv", "dt")
            }

            for i in range(nt):
                fs = slice(i * tile_f, (i + 1) * tile_f)

                prt = iopool.tile([P, 2 * tile_f], f32, tag="pred")
                nc.sync.dma_start(
                    out=prt, in_=pred[:, 2 * i * tile_f : 2 * (i + 1) * tile_f]
                )
                tt = iopool.tile([P, tile_f], f32, tag="target")
                nc.sync.dma_start(out=tt, in_=target[:, fs])
                pt = iopool.tile([P, tile_f], f32, tag="prev")
                nc.sync.dma_start(out=pt, in_=prev[:, fs])
                dtt = iopool.tile([P, tile_f], i32, tag="dt")
                nc.sync.dma_start(out=dtt, in_=dt_p[:, fs])
                pvt = iopool.tile([P, tile_f], i32, tag="pv")
                nc.sync.dma_start(out=pvt, in_=pv_p[:, fs])

                l = prt[:, 0::2]  # lower bounds, stride-2 view
                u = prt[:, 1::2]  # upper bounds

                # DVE gates: absorb the DMA waits into single-dest copies
                # so the STT/TS compute ops ride program order, wait-free.
                def dve_gate(src_ap, tag):
                    return nc.vector.tensor_copy(
                        out=gate_junk[tag], in_=src_ap[:, 0:1]
                    )

                gate_pred = dve_gate(prt, "pred")
                gate_tt = dve_gate(tt, "tt")
                gate_pt = dve_gate(pt, "pt")
                gate_pv = dve_gate(pvt, "pv")
                gate_dt = dve_gate(dtt, "dt")

                # su = l + u, and Ssu column (VectorE)
                su = midpool.tile([P, tile_f], f32, tag="su")
                i_su = nc.vector.scalar_tensor_tensor(
                    out=su, in0=l, scalar=1.0, in1=u,
                    op0=Op.mult, op1=Op.add,
                    accum_out=su_acc[:, i : i + 1],
                )
                add_dep_helper(i_su.ins, gate_pred.ins, False, "gate")
                # max(l,u) junk output, Smx column (VectorE)
                i_mx = nc.vector.scalar_tensor_tensor(
                    out=mxj, in0=l, scalar=1.0, in1=u,
                    op0=Op.mult, op1=Op.max,
                    accum_out=mx_acc[:, i : i + 1],
                )
                add_dep_helper(i_mx.ins, gate_pred.ins, False, "gate")
                # y = 0.5*su - t  (= center - target) (VectorE)
                y = midnr.tile([P, tile_f], f32, tag="y")
                i_y = nc.vector.scalar_tensor_tensor(
                    out=y, in0=su, scalar=0.5, in1=tt,
                    op0=Op.mult, op1=Op.subtract,
                )
                add_dep_helper(i_y.ins, gate_tt.ins, False, "gate")
                # g = 0.5*su - p  (= center - prev) (VectorE)
                g = midpool.tile([P, tile_f], f32, tag="g")
                i_g = nc.vector.scalar_tensor_tensor(
                    out=g, in0=su, scalar=0.5, in1=pt,
                    op0=Op.mult, op1=Op.subtract,
                )
                add_dep_helper(i_g.ins, gate_pt.ins, False, "gate")
                # a = 1 - 2*pv in {-1,+1} (VectorE, 2x tensor_scalar mode)
                a = midpool.tile([P, tile_f], f32, tag="a")
                i_a = nc.vector.tensor_scalar(
                    out=a, in0=pvt, scalar1=-2.0, scalar2=1.0,
                    op0=Op.mult, op1=Op.add,
                )
                add_dep_helper(i_a.ins, gate_pv.ins, False, "gate")
                # s = min(dt,1) * a in {-1,0,1} (VectorE, fused STT)
                s = midpool.tile([P, tile_f], f32, tag="s")
                i_s = nc.vector.scalar_tensor_tensor(
                    out=s, in0=dtt, scalar=1, in1=a,
                    op0=Op.min, op1=Op.mult,
                )
                add_dep_helper(i_s.ins, gate_dt.ins, False, "gate")
                # q = g*s  (= +-(center-prev), masked) (VectorE)
                q = midnr.tile([P, tile_f], f32, tag="q")
                nc.vector.tensor_mul(out=q, in0=g, in1=s)

                # ScalarE gates: single-dest copies absorb the cross-proc
                # waits (DMA for l, DVE for y and q); the dual-dest accum
                # activations then only carry their same-engine WAW wait.
                agp = gatepool.tile([P, 1], f32, tag="agp", name="agp")
                i_agp = nc.scalar.copy(out=agp, in_=prt[:, 0:1])
                agy = gatepool.tile([P, 1], f32, tag="agy", name="agy")
                i_agy = nc.scalar.copy(out=agy, in_=y[:, 0:1])
                agq = gatepool.tile([P, 1], f32, tag="agq", name="agq")
                i_agq = nc.scalar.copy(out=agq, in_=q[:, 0:1])

                # ScalarE: Sl column via Copy-accum on the strided l view
                i_lj = nc.scalar.activation(
                    out=lj, in_=l, func=Act.Copy,
                    accum_out=l_acc[:, i : i + 1],
                )
                add_dep_helper(i_lj.ins, i_agp.ins, False, "gate")
                # ScalarE: Ssq column via Square-accum
                i_sqj = nc.scalar.activation(
                    out=sqj, in_=y, func=Act.Square,
                    accum_out=sq_acc[:, i : i + 1],
                )
                add_dep_helper(i_sqj.ins, i_agy.ins, False, "gate")
                # ScalarE: Spen column via Relu-accum
                i_penj = nc.scalar.activation(
                    out=penj, in_=q, func=Act.Relu,
                    accum_out=pen_acc[:, i : i + 1],
                )
                add_dep_helper(i_penj.ins, i_agq.ins, False, "gate")

            # Stage all partials into one ScalarE-written tile, then a
            # single SWDGE output DMA.  HWDGE out-DMAs carry a queue
            # FIFO-tick wait on top of the compute wait (2 > the pseudo-DMA
            # struct's budget), and one SWDGE DMA adds only one DMASW proc
            # to the kernel-tail drain's wait list (its budget is tight
            # too).  Copy mx first: it is the last DVE-written accumulator,
            # so its single DVE wait covers su_acc as well; the remaining
            # copies and the DMA then ride ScalarE program order.
            stage = accpool.tile([P, 5 * nt], f32, tag="stage")
            nc.scalar.copy(out=stage[:, 2 * nt : 3 * nt], in_=mx_acc)
            nc.scalar.copy(out=stage[:, 0:nt], in_=su_acc)
            nc.scalar.copy(out=stage[:, nt : 2 * nt], in_=l_acc)
            nc.scalar.copy(out=stage[:, 3 * nt : 4 * nt], in_=pen_acc)
            nc.scalar.copy(out=stage[:, 4 * nt : 5 * nt], in_=sq_acc)
            nc.gpsimd.dma_start(out=acc_out[:, :], in_=stage)

    return _legalize_sync_waits(nc)


def make_in_maps(pred, target, prev_pci, delta_time, pv_values):
    """Shard full inputs along the batch axis into 8 per-core input maps."""
    in_maps = []
    for k in range(NCORES):
        sl = slice(k * N, (k + 1) * N)
        in_maps.append(
            {
                "pred": np.ascontiguousarray(pred[sl]).reshape(P, 2 * CPT),
                "target": np.ascontiguousarray(target[sl]).reshape(P, CPT),
                "prev_pci": np.ascontiguousarray(prev_pci[sl]).reshape(P, CPT),
                "delta_time": np.ascontiguousarray(delta_time[sl]).reshape(P, CPT),
                "pv_values": np.ascontiguousarray(pv_values[sl]).reshape(P, CPT),
            }
        )
    return in_maps


def combine_partials(accs, n_total: int) -> np.ndarray:
    """accs: list of per-core [P, 5*NT] partial-sum tensors -> scalar loss."""
    ssu = sl = smx = spen = ssq = 0.0
    for acc in accs:
        a = np.asarray(acc, dtype=np.float64)
        nt = a.shape[1] // 5
        ssu += a[:, 0:nt].sum()
        sl += a[:, nt : 2 * nt].sum()
        smx += a[:, 2 * nt : 3 * nt].sum()
        spen += a[:, 3 * nt : 4 * nt].sum()
        ssq += a[:, 4 * nt : 5 * nt].sum()
    su = ssu - sl
    total = (
        1.5 * ssq + 0.1 * (su - sl) + 10.0 * (smx - su) + 0.5 * spen
    ) / float(n_total)
    return np.array(total, dtype=np.float32)


_PROGRAM = None


def _get_program() -> bass.Bass:
    global _PROGRAM
    if _PROGRAM is None:
        _PROGRAM = build_program()
    return _PROGRAM


def run_on_hw(pred, target, prev_pci, delta_time, pv_values, **runner_kwargs):
    nc = _get_program()
    in_maps = make_in_maps(pred, target, prev_pci, delta_time, pv_values)
    res = run_bass_kernel_spmd(nc, in_maps, list(range(NCORES)), **runner_kwargs)
    accs = [r["acc_out"] for r in res.results]
    return combine_partials(accs, B), res


def kernel(pred, target, prev_pci, delta_time, pv_values) -> np.ndarray:
    pred = np.asarray(pred, dtype=np.float32)
    target = np.asarray(target, dtype=np.float32)
    prev_pci = np.asarray(prev_pci, dtype=np.float32)
    delta_time = np.asarray(delta_time, dtype=np.int32)
    pv_values = np.asarray(pv_values, dtype=np.int32)
    total, _ = run_on_hw(pred, target, prev_pci, delta_time, pv_values)
    return total


# revision 17
# speedup vs baseline: 1.2139x; 1.2139x over previous
"""Trainium2 Bass kernel for nn_CustomLoss_43645457662200.

Loss over B=4,194,304 samples:
    lower = pred[:, 0], upper = pred[:, 1], center = (lower+upper)/2
    center_loss  = mean((target - center)^2)
    width_loss   = mean(upper - lower)
    valid_pen    = mean(relu(lower - upper))
    dir_pen      = sum(relu((center - prev) * s)),  s = (1-2*pv) * (dt != 0)
    total = 1.5*center_loss + 0.1*width_loss + 10*valid_pen + 0.5*dir_pen/B

Strategy: pure data-parallel over 8 NeuronCores (524288 samples each).
Per core, tiles of [128, TILE_F] stream through a fused elementwise
pipeline on the Vector/Scalar engines; all global sums are produced
on-chip via fused accum_out reductions (per-partition [128,1] partials
per tile).  Each core emits a tiny [5,128,NT] partial tensor; the final
combine runs on host in float64 using only sums:
    sum(u-l)      = Ssu - 2*Sl          (Ssu = sum(l+u))
    sum(relu(l-u))= Smx - Su            (Smx = sum(max(l,u)))
    sum((t-c)^2)  = Ssq                 (y = 0.5*(l+u) - t, squared)
    dir_pen       = Spen                (relu((c-p)*s), s = min(dt,1)*(1-2pv))

Sync-wait discipline (walrus codegen rejects instructions whose wait
list exceeds the struct's slot budget — STT/TS, activation-with-accum
and pseudo-DMA all have tiny budgets):
  * every cross-proc dependency of an STT/TS is absorbed by a tiny
    single-dest DVE gate copy ordered in front of it (nosync dep edge);
  * dual-dest activations keep only their same-engine WAW wait; their
    cross-proc waits are absorbed by single-dest ScalarE gate copies;
  * tiles read by a foreign engine live in a no-reuse pool (one buffer
    per iteration) so no WAR wait ever lands on their producer;
  * input tiles also get one buffer per iteration so input DMAs carry
    no WAR waits at all.
"""

import numpy as np

from concourse import bass, mybir
from concourse.bass_utils import run_bass_kernel_spmd
from concourse.tile import TileContext
from concourse.tile_rust import add_dep_helper

B = 4_194_304
NCORES = 8
N = B // NCORES  # 524288 samples per core
P = 128
CPT = N // P  # 4096 free-dim columns per core per tensor
TILE_F = 1024

f32 = mybir.dt.float32
i32 = mybir.dt.int32


def _legalize_sync_waits(nc: bass.Bass) -> bass.Bass:
    """Split multi-wait instructions for this walrus build.

    The neuronxcc walrus in this container rejects ANY instruction whose
    sync_info carries more than one wait command ("Too many sync wait
    commands", even for plain TensorTensor — the stock tile_nary_add
    kernel trips it too).  Hoist all but the last wait of each
    instruction onto freshly injected same-engine NoOps placed directly
    before it; engine sequencers execute waits in stream order, so the
    semantics are identical.
    """
    counter = 0
    for fn in nc.m.functions:
        for blk in fn.blocks:
            insts = blk.instructions
            out = []
            changed = False
            for ins in insts:
                si = ins.sync_info
                waits = list(si.on_wait) if si is not None and si.on_wait else []
                if len(waits) > 1:
                    changed = True
                    for w in waits[:-1]:
                        counter += 1
                        nop = mybir.InstNoOp(name=f"waitsplit_{counter}")
                        nop.engine = ins.engine
                        nop.sync_info = mybir.SyncInfo(on_wait=[w], on_update=[])
                        out.append(nop)
                    ins.sync_info = mybir.SyncInfo(
                        on_wait=[waits[-1]], on_update=list(si.on_update or [])
                    )
                out.append(ins)
            if changed:
                blk.instructions = out
    return nc


def build_program(cpt: int = CPT, tile_f: int = TILE_F, legalize: bool = True) -> bass.Bass:
    assert cpt % tile_f == 0
    nt = cpt // tile_f
    Op = mybir.AluOpType
    Act = mybir.ActivationFunctionType

    nc = bass.Bass()
    # All five tensors are host-packed into one interleaved array so each
    # tile is ONE DMA whose per-partition runs are 6*tile_f*4 bytes of
    # contiguous DRAM.  The per-row layout of each 6F-column tile block:
    #   [ target(F) | prev(F) | dt(F as f32 bits) | pv(F) | pred(2F) ]
    # Small per-partition descriptors (4-8KB) were the bottleneck: the 16
    # SDMA engines spent ~250ns per descriptor, latency-bound at ~60%
    # occupancy (~217GB/s effective of the ~358GB/s per-core HBM peak).
    packed = nc.declare_dram_parameter(
        "packed", [P, nt * 6 * tile_f], f32, isOutput=False
    )
    acc_out = nc.declare_dram_parameter("acc_out", [P, 5 * nt], f32, isOutput=True)

    with TileContext(nc) as tc:
        with (
            tc.tile_pool(name="accs", bufs=1) as accpool,
            tc.tile_pool(name="io", bufs=max(nt, 2)) as iopool,
            tc.tile_pool(name="mid", bufs=2) as midpool,
            tc.tile_pool(name="mid_nr", bufs=max(nt, 2)) as midnr,
            tc.tile_pool(name="gates", bufs=max(nt, 2)) as gatepool,
        ):
            su_acc = accpool.tile([P, nt], f32, tag="su_acc")
            l_acc = accpool.tile([P, nt], f32, tag="l_acc")
            mx_acc = accpool.tile([P, nt], f32, tag="mx_acc")
            pen_acc = accpool.tile([P, nt], f32, tag="pen_acc")
            sq_acc = accpool.tile([P, nt], f32, tag="sq_acc")

            # Persistent junk sinks for the dual-dest ops whose primary
            # output is unused.  Persistent => cross-iteration WAW stays
            # same-engine program order (DVE emits no wait for it; the
            # ScalarE ops pay exactly one same-engine wait, which is
            # within the activation struct's budget).
            mxj = accpool.tile([P, tile_f], f32, tag="mxj")
            lj = accpool.tile([P, tile_f], f32, tag="lj")
            sqj = accpool.tile([P, tile_f], f32, tag="sqj")
            penj = accpool.tile([P, tile_f], f32, tag="penj")
            gate_junk = {
                tag: accpool.tile([P, 1], f32, tag=f"gj_{tag}", name=f"gj_{tag}")
                for tag in ("pred",)
            }

            for i in range(nt):
                F = tile_f
                pk = iopool.tile([P, 6 * F], f32, tag="pk")
                nc.sync.dma_start(
                    out=pk, in_=packed[:, i * 6 * F : (i + 1) * 6 * F]
                )

                tt = pk[:, 0:F]
                pt = pk[:, F : 2 * F]
                dtt = pk[:, 2 * F : 3 * F].bitcast(i32)
                pvt = pk[:, 3 * F : 4 * F].bitcast(i32)
                predv = pk[:, 4 * F : 6 * F]
                l = predv[:, 0::2]  # lower bounds, stride-2 view
                u = predv[:, 1::2]  # upper bounds

                # DVE gate: absorb the DMA wait into a single-dest copy
                # so the STT/TS compute ops ride program order, wait-free.
                gate_pk = nc.vector.tensor_copy(
                    out=gate_junk["pred"], in_=pk[:, 0:1]
                )
                gate_pred = gate_tt = gate_pt = gate_pv = gate_dt = gate_pk

                # su = l + u, and Ssu column (VectorE)
                su = midpool.tile([P, tile_f], f32, tag="su")
                i_su = nc.vector.scalar_tensor_tensor(
                    out=su, in0=l, scalar=1.0, in1=u,
                    op0=Op.mult, op1=Op.add,
                    accum_out=su_acc[:, i : i + 1],
                )
                add_dep_helper(i_su.ins, gate_pred.ins, False, "gate")
                # max(l,u) junk output, Smx column (VectorE)
                i_mx = nc.vector.scalar_tensor_tensor(
                    out=mxj, in0=l, scalar=1.0, in1=u,
                    op0=Op.mult, op1=Op.max,
                    accum_out=mx_acc[:, i : i + 1],
                )
                add_dep_helper(i_mx.ins, gate_pred.ins, False, "gate")
                # y = 0.5*su - t  (= center - target) (VectorE)
                y = midnr.tile([P, tile_f], f32, tag="y")
                i_y = nc.vector.scalar_tensor_tensor(
                    out=y, in0=su, scalar=0.5, in1=tt,
                    op0=Op.mult, op1=Op.subtract,
                )
                add_dep_helper(i_y.ins, gate_tt.ins, False, "gate")
                # g = 0.5*su - p  (= center - prev) (VectorE)
                g = midpool.tile([P, tile_f], f32, tag="g")
                i_g = nc.vector.scalar_tensor_tensor(
                    out=g, in0=su, scalar=0.5, in1=pt,
                    op0=Op.mult, op1=Op.subtract,
                )
                add_dep_helper(i_g.ins, gate_pt.ins, False, "gate")
                # a = 1 - 2*pv in {-1,+1} (VectorE, 2x tensor_scalar mode)
                a = midpool.tile([P, tile_f], f32, tag="a")
                i_a = nc.vector.tensor_scalar(
                    out=a, in0=pvt, scalar1=-2.0, scalar2=1.0,
                    op0=Op.mult, op1=Op.add,
                )
                add_dep_helper(i_a.ins, gate_pv.ins, False, "gate")
                # s = min(dt,1) * a in {-1,0,1} (VectorE, fused STT)
                s = midpool.tile([P, tile_f], f32, tag="s")
                i_s = nc.vector.scalar_tensor_tensor(
                    out=s, in0=dtt, scalar=1, in1=a,
                    op0=Op.min, op1=Op.mult,
                )
                add_dep_helper(i_s.ins, gate_dt.ins, False, "gate")
                # q = g*s  (= +-(center-prev), masked) (VectorE)
                q = midnr.tile([P, tile_f], f32, tag="q")
                nc.vector.tensor_mul(out=q, in0=g, in1=s)

                # ScalarE gates: single-dest copies absorb the cross-proc
                # waits (DMA for l, DVE for y and q); the dual-dest accum
                # activations then only carry their same-engine WAW wait.
                agp = gatepool.tile([P, 1], f32, tag="agp", name="agp")
                i_agp = nc.scalar.copy(out=agp, in_=pk[:, 0:1])
                agy = gatepool.tile([P, 1], f32, tag="agy", name="agy")
                i_agy = nc.scalar.copy(out=agy, in_=y[:, 0:1])
                agq = gatepool.tile([P, 1], f32, tag="agq", name="agq")
                i_agq = nc.scalar.copy(out=agq, in_=q[:, 0:1])

                # ScalarE: Sl column via Copy-accum on the strided l view
                i_lj = nc.scalar.activation(
                    out=lj, in_=l, func=Act.Copy,
                    accum_out=l_acc[:, i : i + 1],
                )
                add_dep_helper(i_lj.ins, i_agp.ins, False, "gate")
                # ScalarE: Ssq column via Square-accum
                i_sqj = nc.scalar.activation(
                    out=sqj, in_=y, func=Act.Square,
                    accum_out=sq_acc[:, i : i + 1],
                )
                add_dep_helper(i_sqj.ins, i_agy.ins, False, "gate")
                # ScalarE: Spen column via Relu-accum
                i_penj = nc.scalar.activation(
                    out=penj, in_=q, func=Act.Relu,
                    accum_out=pen_acc[:, i : i + 1],
                )
                add_dep_helper(i_penj.ins, i_agq.ins, False, "gate")

            # Stage all partials into one ScalarE-written tile, then a
            # single SWDGE output DMA.  HWDGE out-DMAs carry a queue
            # FIFO-tick wait on top of the compute wait (2 > the pseudo-DMA
            # struct's budget), and one SWDGE DMA adds only one DMASW proc
            # to the kernel-tail drain's wait list (its budget is tight
            # too).  Copy mx first: it is the last DVE-written accumulator,
            # so its single DVE wait covers su_acc as well; the remaining
            # copies and the DMA then ride ScalarE program order.
            stage = accpool.tile([P, 5 * nt], f32, tag="stage")
            nc.scalar.copy(out=stage[:, 2 * nt : 3 * nt], in_=mx_acc)
            nc.scalar.copy(out=stage[:, 0:nt], in_=su_acc)
            nc.scalar.copy(out=stage[:, nt : 2 * nt], in_=l_acc)
            nc.scalar.copy(out=stage[:, 3 * nt : 4 * nt], in_=pen_acc)
            nc.scalar.copy(out=stage[:, 4 * nt : 5 * nt], in_=sq_acc)
            nc.gpsimd.dma_start(out=acc_out[:, :], in_=stage)

    return _legalize_sync_waits(nc) if legalize else nc


def pack_arrays(pred2, t2, p2, dt2, pv2, tile_f):
    """Interleave per-core [P, cpt]-shaped tensors into the packed layout.

    Per tile block of 6*tile_f columns:
      [ target(F) | prev(F) | dt bits(F) | pv bits(F) | pred(2F) ]
    """
    cptl = t2.shape[1]
    ntl = cptl // tile_f
    parts = [
        t2.reshape(P, ntl, tile_f),
        p2.reshape(P, ntl, tile_f),
        np.ascontiguousarray(dt2.reshape(P, ntl, tile_f)).view(np.float32),
        np.ascontiguousarray(pv2.reshape(P, ntl, tile_f)).view(np.float32),
        pred2.reshape(P, ntl, 2 * tile_f),
    ]
    return np.concatenate(parts, axis=2).reshape(P, ntl * 6 * tile_f)


def make_in_maps(pred, target, prev_pci, delta_time, pv_values):
    """Shard full inputs along the batch axis into 8 per-core input maps."""
    in_maps = []
    for k in range(NCORES):
        sl = slice(k * N, (k + 1) * N)
        in_maps.append(
            {
                "packed": pack_arrays(
                    np.ascontiguousarray(pred[sl]).reshape(P, 2 * CPT),
                    np.ascontiguousarray(target[sl]).reshape(P, CPT),
                    np.ascontiguousarray(prev_pci[sl]).reshape(P, CPT),
                    np.ascontiguousarray(delta_time[sl]).reshape(P, CPT),
                    np.ascontiguousarray(pv_values[sl]).reshape(P, CPT),
                    TILE_F,
                )
            }
        )
    return in_maps


def combine_partials(accs, n_total: int) -> np.ndarray:
    """accs: list of per-core [P, 5*NT] partial-sum tensors -> scalar loss."""
    ssu = sl = smx = spen = ssq = 0.0
    for acc in accs:
        a = np.asarray(acc, dtype=np.float64)
        nt = a.shape[1] // 5
        ssu += a[:, 0:nt].sum()
        sl += a[:, nt : 2 * nt].sum()
        smx += a[:, 2 * nt : 3 * nt].sum()
        spen += a[:, 3 * nt : 4 * nt].sum()
        ssq += a[:, 4 * nt : 5 * nt].sum()
    su = ssu - sl
    total = (
        1.5 * ssq + 0.1 * (su - sl) + 10.0 * (smx - su) + 0.5 * spen
    ) / float(n_total)
    return np.array(total, dtype=np.float32)


_PROGRAM = None


def _get_program() -> bass.Bass:
    global _PROGRAM
    if _PROGRAM is None:
        _PROGRAM = build_program()
    return _PROGRAM


def run_on_hw(pred, target, prev_pci, delta_time, pv_values, **runner_kwargs):
    nc = _get_program()
    in_maps = make_in_maps(pred, target, prev_pci, delta_time, pv_values)
    res = run_bass_kernel_spmd(nc, in_maps, list(range(NCORES)), **runner_kwargs)
    accs = [r["acc_out"] for r in res.results]
    return combine_partials(accs, B), res


def kernel(pred, target, prev_pci, delta_time, pv_values) -> np.ndarray:
    pred = np.asarray(pred, dtype=np.float32)
    target = np.asarray(target, dtype=np.float32)
    prev_pci = np.asarray(prev_pci, dtype=np.float32)
    delta_time = np.asarray(delta_time, dtype=np.int32)
    pv_values = np.asarray(pv_values, dtype=np.int32)
    total, _ = run_on_hw(pred, target, prev_pci, delta_time, pv_values)
    return total
